# revision 1
# baseline (speedup 1.0000x reference)
import numpy as np

P = 128
B = 4
L = 1024
DIN = 32
D = 512
E = 512          # local half of d_inner per core
N = 16
KC = 4
R = 32
NL = 4
EPS = 1e-5
RG = [[0, 1], [2, 3], [4, 5], [6, 7]]


class _FI:
    def then_inc(self, *a, **k):
        return self


class _FE:
    def __getattr__(self, name):
        return lambda *a, **k: _FI()


def _prep(inputs, c):
    g = lambda k: np.asarray(inputs[k], np.float32)
    b, hf = c // 2, c % 2
    es = slice(hf * E, (hf + 1) * E)
    m = {}
    m["xT"] = np.ascontiguousarray(g("x")[b].T)                      # (32,1024)
    m["f1"] = np.ascontiguousarray(g("fc1_w").T)                     # (32,512)
    m["f1b"] = np.ascontiguousarray(g("fc1_b").reshape(4, P).T)      # (128,4)
    m["f2"] = np.ascontiguousarray(g("fc2_w").reshape(4, P).T)       # (128,4)
    m["f2b"] = np.array([[-float(g("fc2_b")[0])]], np.float32)
    for i in range(NL):
        W = g("in_proj_w")[i]
        Wl = np.concatenate([W[hf * E:(hf + 1) * E], W[1024 + hf * E:1024 + (hf + 1) * E]], 0)
        Wl = Wl * g("norm_w")[i][None, :]
        lt = Wl.T                                                    # (512,1024)
        m[f"wi{i}"] = np.ascontiguousarray(
            np.concatenate([lt[k * P:(k + 1) * P] for k in range(4)], 1))   # (128,4096)
        lx = g("xproj_w")[i][:, es].T                                # (512,64)
        m[f"wx{i}"] = np.ascontiguousarray(
            np.concatenate([lx[k * P:(k + 1) * P] for k in range(4)], 1))   # (128,256)
        m[f"wd{i}"] = np.ascontiguousarray(g("dtproj_w")[i][es].T)   # (32,512)
        lo = g("out_proj_w")[i][:, es].T                             # (512,512)
        m[f"wo{i}"] = np.ascontiguousarray(
            np.concatenate([lo[k * P:(k + 1) * P] for k in range(4)], 1))   # (128,2048)
        A = -np.exp(g("A_log")[i][es])                               # (512,16)
        cp = np.zeros((P, 92), np.float32)
        for k in range(4):
            cp[:, k * 16:(k + 1) * 16] = A[k * P:(k + 1) * P]
        cp[:, 64:68] = g("D_param")[i][es].reshape(4, P).T
        cw = g("conv_w")[i][es]                                      # (512,4)
        for k in range(4):
            cp[:, 68 + k * 4:68 + (k + 1) * 4] = cw[k * P:(k + 1) * P]
        cp[:, 84:88] = g("conv_b")[i][es].reshape(4, P).T
        cp[:, 88:92] = g("dtproj_b")[i][es].reshape(4, P).T
        m[f"cp{i}"] = cp
    return m


def _build(bass, mybir):
    from contextlib import ExitStack
    AF = mybir.ActivationFunctionType
    AO = mybir.AluOpType
    mult, add = AO.mult, AO.add
    f32 = mybir.dt.float32
    nc = bass.Bass(num_devices=8)
    for cv in (EPS, 1.0 / D, -0.5, -1.0):
        t = nc.alloc_sbuf_tensor(f"cst-{cv}", [P, 1], f32)
        nc.gpsimd.memset(t.ap(), cv)
        nc.const_aps.aps[(f32, cv)] = t.ap()

    din = lambda n, s: nc.dram_tensor(n, s, f32, kind="ExternalInput")
    xT_d = din("xT", [32, L]); f1_d = din("f1", [32, D])
    f1b_d = din("f1b", [P, 4]); f2_d = din("f2", [P, 4]); f2b_d = din("f2b", [1, 1])
    wi_d = [din(f"wi{i}", [P, 4096]) for i in range(NL)]
    wx_d = [din(f"wx{i}", [P, 256]) for i in range(NL)]
    wd_d = [din(f"wd{i}", [32, D]) for i in range(NL)]
    wo_d = [din(f"wo{i}", [P, 2048]) for i in range(NL)]
    cp_d = [din(f"cp{i}", [P, 92]) for i in range(NL)]
    out_d = nc.dram_tensor("out", [1, L], f32, kind="ExternalOutput")

    ocol_d = nc.inline_tensor(np.ones((P, 1), np.float32), name="ocol")
    orow_d = nc.inline_tensor(np.ones((1, P), np.float32), name="orow")
    oh = np.zeros((16, 16 * P), np.float32)
    for n in range(16):
        oh[n, n * P:(n + 1) * P] = 1.0
    oh_d = nc.inline_tensor(oh, name="oh16")

    cc1i = nc.dram_tensor("cc1i", [64, L], f32, kind="Internal")
    cc1o = nc.dram_tensor("cc1o", [64, L], f32, kind="Internal")
    cc2i = nc.dram_tensor("cc2i", [D, L], f32, kind="Internal")
    cc2o = nc.dram_tensor("cc2o", [D, L], f32, kind="Internal")

    es = ExitStack()
    block = es.enter_context(nc.Block())
    SEd = es.enter_context(nc.semaphore("dsem"))
    SEp = es.enter_context(nc.semaphore("psem"))
    SEa = es.enter_context(nc.semaphore("asem"))
    SEv = es.enter_context(nc.semaphore("vsem"))
    SEg = es.enter_context(nc.semaphore("gsem"))
    sb = lambda n, s: es.enter_context(nc.sbuf_tensor(n, s, f32))
    pt = lambda n, s: es.enter_context(nc.psum_tensor(n, s, f32))

    xT = sb("xT_s", [32, L]); f1 = sb("f1_s", [32, D])
    f1b = sb("f1b_s", [P, 4]); f2 = sb("f2_s", [P, 4]); f2b = sb("f2b_s", [1, 1])
    ocol = sb("ocol_s", [P, 1]); orow = sb("orow_s", [1, P]); oh16 = sb("oh16_s", [16, 16 * P])
    wi = sb("wi_s", [P, 4096]); wx = sb("wx_s", [P, 256]); wd = sb("wd_s", [32, D])
    wo = sb("wo_s", [P, 2048]); cp = sb("cp_s", [P, 92])
    h = sb("h_s", [P, 4096]); xn = sb("xn_s", [P, 4096])
    xpad = [sb(f"xp{k}_s", [P, 1028]) for k in range(4)]
    co = sb("co_s", [P, 4096]); z = sb("z_s", [P, 4096])
    dl = sb("dl_s", [P, 4096]); ya = sb("ya_s", [P, 4096])
    dApp = sb("dApp_s", [P, 2048]); st = sb("st_s", [P, L]); yn = sb("yn_s", [P, L])
    dt32 = sb("dt32_s", [32, L]); b16 = sb("b16_s", [16, L]); c16 = sb("c16_s", [16, L])
    dbcp = sb("dbcp_s", [64, L]); row = sb("row_s", [1, L])

    pxA = pt("pxA", [P, L]); pxB = pt("pxB", [P, L])
    pB = pt("pB", [P, L]); pC = pt("pC", [P, L])

    def prog(s, p, a, v, g):
        dct = [0]; pct = [0]; act = [0]; vct = [0]; gct = [0]

        def DS(out, in_):
            s.dma_start(out=out, in_=in_).then_inc(SEd, 16)
            dct[0] += 1
        def GD(out, in_):
            g.dma_start(out=out, in_=in_).then_inc(SEg, 16)
            gct[0] += 16
        def MM(out, lhsT, rhs, start, stop, inc=False):
            i = p.matmul(out, lhsT, rhs, start=start, stop=stop)
            if inc:
                i.then_inc(SEp, 1); pct[0] += 1
        def ACT(out, in_, fn, inc=False, **kw):
            i = a.activation(out, in_, fn, **kw)
            if inc:
                i.then_inc(SEa, 1); act[0] += 1
        def vinc(i):
            i.then_inc(SEv, 1); vct[0] += 1

        mt = lambda t, m: t[:, m * L:(m + 1) * L]
        fs = lambda f: slice(f * 512, (f + 1) * 512)

        # ---- prologue DMAs
        for dst, src in [(xT, xT_d), (f1, f1_d), (f1b, f1b_d), (f2, f2_d),
                         (f2b, f2b_d), (ocol, ocol_d), (orow, orow_d), (oh16, oh_d)]:
            DS(dst[:], src[:])
        # layer 0 weights
        for dst, src in [(wi, wi_d[0]), (wx, wx_d[0]), (wd, wd_d[0]),
                         (wo, wo_d[0]), (cp, cp_d[0])]:
            DS(dst[:], src[:])
        dall = 16 * dct[0]

        # ---- fc1: h = fc1_w @ xT + fc1_b
        p.wait_ge(SEd, dall)
        a.wait_ge(SEd, dall)
        v.wait_ge(SEd, dall)
        for k in range(4):
            v.memset(xpad[k][:, 0:3], 0.0)
        aev = {}
        for m in range(4):
            buf = pxA if m % 2 == 0 else pxB
            if m >= 2:
                p.wait_ge(SEa, aev[m - 2])
            MM(buf[:, fs(0)], f1[:, m * P:(m + 1) * P], xT[:, fs(0)], True, True)
            MM(buf[:, fs(1)], f1[:, m * P:(m + 1) * P], xT[:, fs(1)], True, True, inc=True)
            pm = pct[0]
            a.wait_ge(SEp, pm)
            ACT(mt(h, m), buf[:], AF.Identity, bias=f1b[:, m:m + 1], inc=True)
            aev[m] = act[0]

        # ---- layers
        for i in range(NL):
            if i > 0:
                s.wait_ge(SEv, vct[0])
                for dst, src in [(wi, wi_d[i]), (wx, wx_d[i]), (wd, wd_d[i]),
                                 (wo, wo_d[i]), (cp, cp_d[i])]:
                    DS(dst[:], src[:])
                dall = 16 * dct[0]
                p.wait_ge(SEd, dall)
                a.wait_ge(SEd, dall)
                v.wait_ge(SEd, dall)
                a.wait_ge(SEv, vct[0])

            # A: rmsnorm -> xn
            for m in range(4):
                ACT(mt(xn, m), mt(h, m), AF.Square, inc=(m == 3))
            a_sq = act[0]
            p.wait_ge(SEa, a_sq)
            for m in range(4):
                for f in range(2):
                    MM(pxA[0:1, fs(f)], ocol[:], xn[:, m * L + f * 512: m * L + (f + 1) * 512],
                       m == 0, m == 3, inc=(m == 3 and f == 1))
            a.wait_ge(SEp, pct[0])
            ACT(row[:], pxA[0:1, :], AF.Ln, scale=1.0 / D, bias=EPS)
            ACT(row[:], row[:], AF.Exp, scale=-0.5, inc=True)
            a_rs = act[0]
            p.wait_ge(SEa, a_rs)
            for f in range(2):
                MM(pB[:, fs(f)], orow[:], row[0:1, fs(f)], True, True, inc=(f == 1))
            v.wait_ge(SEp, pct[0])
            for m in range(4):
                last = v.tensor_tensor(mt(xn, m), mt(h, m), pB[:], mult)
            vinc(last)
            v_xn = vct[0]

            # B: in_proj -> xpad (x part), z
            p.wait_ge(SEv, v_xn)
            aev = {}
            for m in range(8):
                buf = pxA if m % 2 == 0 else pxB
                if m >= 2:
                    p.wait_ge(SEa, aev[m - 2])
                for f in range(2):
                    for k in range(4):
                        MM(buf[:, fs(f)], wi[:, k * L + m * P: k * L + (m + 1) * P],
                           xn[:, k * L + f * 512: k * L + (f + 1) * 512],
                           k == 0, k == 3, inc=(f == 1 and k == 3))
                a.wait_ge(SEp, pct[0])
                dst = xpad[m][:, 3:3 + L] if m < 4 else mt(z, m - 4)
                ACT(dst, buf[:], AF.Identity, inc=True)
                aev[m] = act[0]

            # C: conv + silu -> co
            v.wait_ge(SEa, aev[3])
            for k in range(4):
                wc = lambda j: cp[:, 68 + k * 4 + j: 68 + k * 4 + j + 1]
                bc = cp[:, 84 + k:85 + k]
                v.tensor_scalar(mt(co, k), xpad[k][:, 0:L], wc(0), bc, mult, add)
                for j in range(1, 4):
                    last = v.scalar_tensor_tensor(mt(co, k), xpad[k][:, j:j + L], wc(j),
                                                  mt(co, k), mult, add)
            vinc(last)
            v_conv = vct[0]
            a.wait_ge(SEa, aev[7])  # no-op ordering aid
            a.wait_ge(SEv, v_conv)
            a_sg = {}
            for k in range(4):
                sg = mt(xn, k)
                ACT(sg, mt(co, k), AF.Exp, scale=-1.0)
                ACT(sg, sg, AF.Ln, bias=1.0)
                ACT(sg, sg, AF.Exp, scale=-1.0, inc=True)
                a_sg[k] = act[0]
            for k in range(4):
                v.wait_ge(SEa, a_sg[k])
                last = v.tensor_tensor(mt(co, k), mt(co, k), mt(xn, k), mult)
            vinc(last)
            v_silu = vct[0]

            # D: xproj -> dbc partial, AllReduce pair, split dt/B/C
            p.wait_ge(SEv, v_silu)
            for f in range(2):
                for k in range(4):
                    MM(pxA[0:64, fs(f)], wx[:, k * 64:(k + 1) * 64],
                       co[:, k * L + f * 512: k * L + (f + 1) * 512],
                       k == 0, k == 3, inc=(f == 1 and k == 3))
            a.wait_ge(SEp, pct[0])
            ACT(dbcp[:], pxA[0:64, :], AF.Identity, inc=True)
            g.wait_ge(SEa, act[0])
            GD(cc1i[:], dbcp[:])
            g.wait_ge(SEg, gct[0])
            g.collective_compute("AllReduce", add, replica_groups=RG,
                                 ins=[cc1i[:]], outs=[cc1o[:]]).then_inc(SEg, 1)
            gct[0] += 1
            g.wait_ge(SEg, gct[0])
            GD(dt32[:], cc1o[0:32, :])
            GD(b16[:], cc1o[32:48, :])
            GD(c16[:], cc1o[48:64, :])
            g_dbc = gct[0]

            # E: dtproj + softplus -> dl
            p.wait_ge(SEg, g_dbc)
            aev2 = {}
            for m in range(4):
                buf = pxA if m % 2 == 0 else pxB
                if m >= 2:
                    p.wait_ge(SEa, aev2[m - 2])
                for f in range(2):
                    MM(buf[:, fs(f)], wd[:, m * P:(m + 1) * P], dt32[:, fs(f)],
                       True, True, inc=(f == 1))
                a.wait_ge(SEp, pct[0])
                a.wait_ge(SEg, g_dbc)  # dt32 also read by ACT? no; ordering aid
                ACT(mt(dl, m), buf[:], AF.Exp, bias=cp[:, 88 + m:89 + m])
                ACT(mt(dl, m), mt(dl, m), AF.Ln, bias=1.0, inc=True)
                aev2[m] = act[0]

            # F: scan
            v.wait_ge(SEa, aev2[3])
            for k in range(4):
                last = v.tensor_tensor(xpad[k][:, 3:3 + L], mt(dl, k), mt(co, k), mult)
            vinc(last)
            p.wait_ge(SEa, act[0])
            p.wait_ge(SEv, vct[0])
            p_n = {}; v_it = {}; a_it = {}; v_n = {}
            for n in range(16):
                Bs = pB if n % 2 == 0 else pxA
                Cs = pC if n % 2 == 0 else pxB
                if n >= 2:
                    p.wait_ge(SEv, v_n[n - 2])
                for f in range(2):
                    MM(Bs[:, fs(f)], oh16[:, n * P:(n + 1) * P], b16[:, fs(f)], True, True)
                for f in range(2):
                    MM(Cs[:, fs(f)], oh16[:, n * P:(n + 1) * P], c16[:, fs(f)], True, True,
                       inc=(f == 1))
                p_n[n] = pct[0]
                for k in range(4):
                    it = n * 4 + k
                    sl = dApp[:, (it % 2) * L:((it % 2) + 1) * L]
                    if it >= 2:
                        a.wait_ge(SEv, v_it[it - 2])
                    ACT(sl, mt(dl, k), AF.Exp, scale=cp[:, k * 16 + n: k * 16 + n + 1],
                        inc=True)
                    a_it[it] = act[0]
                    if k == 0:
                        v.wait_ge(SEp, p_n[n])
                    v.wait_ge(SEa, a_it[it])
                    v.tensor_tensor(st[:], xpad[k][:, 3:3 + L], Bs[:], mult)
                    v.tensor_tensor_scan(st[:], sl, st[:], 0.0, mult, add)
                    if n == 0:
                        last = v.tensor_tensor(mt(ya, k), st[:], Cs[:], mult)
                    else:
                        v.tensor_tensor(yn[:], st[:], Cs[:], mult)
                        last = v.tensor_tensor(mt(ya, k), mt(ya, k), yn[:], add)
                    vinc(last)
                    v_it[it] = vct[0]
                v_n[n] = vct[0]

            # G: y = (ya + co*D) * silu(z); out_proj; AllReduce; residual
            for k in range(4):
                last = v.scalar_tensor_tensor(mt(ya, k), mt(co, k), cp[:, 64 + k:65 + k],
                                              mt(ya, k), mult, add)
            vinc(last)
            a.wait_ge(SEv, vct[0])
            a_sz = {}
            for k in range(4):
                sg = mt(xn, k)
                ACT(sg, mt(z, k), AF.Exp, scale=-1.0)
                ACT(sg, sg, AF.Ln, bias=1.0)
                ACT(sg, sg, AF.Exp, scale=-1.0, inc=True)
                a_sz[k] = act[0]
            for k in range(4):
                v.wait_ge(SEa, a_sz[k])
                v.tensor_tensor(mt(z, k), mt(z, k), mt(xn, k), mult)
                last = v.tensor_tensor(mt(ya, k), mt(ya, k), mt(z, k), mult)
            vinc(last)
            v_y = vct[0]
            p.wait_ge(SEv, v_y)
            aop = {}
            for m in range(4):
                buf = pxA if m % 2 == 0 else pxB
                if m >= 2:
                    p.wait_ge(SEa, aop[m - 2])
                for f in range(2):
                    for k in range(4):
                        MM(buf[:, fs(f)], wo[:, k * 512 + m * P: k * 512 + (m + 1) * P],
                           ya[:, k * L + f * 512: k * L + (f + 1) * 512],
                           k == 0, k == 3, inc=(f == 1 and k == 3))
                a.wait_ge(SEp, pct[0])
                ACT(mt(co, m), buf[:], AF.Identity, inc=True)
                aop[m] = act[0]
            g.wait_ge(SEa, aop[3])
            for m in range(4):
                GD(cc2i[m * P:(m + 1) * P, :], mt(co, m))
            g.wait_ge(SEg, gct[0])
            g.collective_compute("AllReduce", add, replica_groups=RG,
                                 ins=[cc2i[:]], outs=[cc2o[:]]).then_inc(SEg, 1)
            gct[0] += 1
            g.wait_ge(SEg, gct[0])
            for m in range(4):
                GD(mt(z, m), cc2o[m * P:(m + 1) * P, :])
            v.wait_ge(SEg, gct[0])
            for k in range(4):
                last = v.tensor_tensor(mt(h, k), mt(h, k), mt(z, k), add)
            vinc(last)

        # ---- fc2 + sigmoid
        p.wait_ge(SEv, vct[0])
        for f in range(2):
            for k in range(4):
                MM(pxA[0:1, fs(f)], f2[:, k:k + 1],
                   h[:, k * L + f * 512: k * L + (f + 1) * 512],
                   k == 0, k == 3, inc=(f == 1 and k == 3))
        a.wait_ge(SEp, pct[0])
        ACT(row[:], pxA[0:1, :], AF.Exp, scale=-1.0, bias=f2b[0:1, 0:1])
        ACT(row[:], row[:], AF.Ln, bias=1.0)
        ACT(row[:], row[:], AF.Exp, scale=-1.0, inc=True)
        s.wait_ge(SEa, act[0])
        DS(out_d[:], row[:])
        s.wait_ge(SEd, 16 * dct[0])

    @block.sync
    def _(s):
        prog(s, _FE(), _FE(), _FE(), _FE())

    @block.tensor
    def _(p):
        prog(_FE(), p, _FE(), _FE(), _FE())

    @block.scalar
    def _(a):
        prog(_FE(), _FE(), a, _FE(), _FE())

    @block.vector
    def _(v):
        prog(_FE(), _FE(), _FE(), v, _FE())

    @block.gpsimd
    def _(g):
        prog(_FE(), _FE(), _FE(), _FE(), g)

    es.close()
    return nc


def kernel(**inputs):
    import concourse.bass as bass
    import concourse.bass_utils as bum
    from concourse import mybir
    from concourse.bass_utils import run_bass_kernel_spmd
    bum.upload_artifacts = lambda t: t
    nc = _build(bass, mybir)
    in_maps = [_prep(inputs, c) for c in range(8)]
    res = run_bass_kernel_spmd(nc, in_maps, list(range(8)), trace=False)
    out = np.zeros((B * L,), np.float32)
    for b in range(B):
        out[b * L:(b + 1) * L] = np.asarray(res.results[2 * b]["out"], np.float32).reshape(-1)
    return out

